# revision 43
# baseline (speedup 1.0000x reference)
"""Trainium2 Bass kernel for the actor-critic LSTM agent (nn_Agent_37984690765937).

Strategy:
  - 8 cores, 4+4 split: cores 0-3 run the ACTOR branch, cores 4-7 the CRITIC
    branch; each core owns a block of 64 envs (B=256 = 4 blocks x 64).
    Both branches run the identical SPMD program; only the weights/inputs
    fed to each core differ.  Actor cores' value output is discarded by the
    host, critic cores' logp/entropy likewise.
  - Everything on-chip is kept in TRANSPOSED (feature-major) layout so that
    activations are always matmul-ready ([K-chunks x 128, cols]); the moving
    operand of every matmul is streamed with the batch/cols on the free dim.
  - Per-core pipeline over 8 blocks of 16 timesteps (1024 cols):
      MLP1 -> MLP2 -> X-gates precompute (bf16 matmuls, fp32 psum)
      -> 16 masked-LSTM steps (h stationary bf16, Whh moving bf16, c fp32)
      -> per-2-step head matmuls (logits+value fused [*,19])
    Tile's scheduler overlaps block b+1's bulk matmuls with block b's LSTM.
  - Softmax / log / entropy / action-gather are deferred to one batched
    final phase (exp/ln live in a different ACT table set than sigmoid/tanh,
    so interleaving would thrash the activation tables).
"""

import sys

if "/opt/trn_rl_repo" not in sys.path:
    sys.path.insert(0, "/opt/trn_rl_repo")

import numpy as np

import concourse.bass as bass
import concourse.tile as tile
from concourse import mybir
from concourse.bass_utils import run_bass_kernel_spmd

AF = mybir.ActivationFunctionType
ALU = mybir.AluOpType
F32 = mybir.dt.float32
BF16 = mybir.dt.bfloat16
I32 = mybir.dt.int32

OBS, ACTN, H, T, B = 128, 18, 512, 128, 256
NCORES = 8
BS = 64          # envs per core
KO = OBS // 128  # 1
KH = H // 128    # 4
MG = (4 * H) // 128  # 16 gate chunks
NHD = ACTN + 1   # fused logits+value head width


def split_wide_waits(nc, max_waits=1):
    """This walrus build supports only one sync-wait per instruction; move
    extra waits onto preceding single-wait NoOps on the same engine."""
    n = 0
    for f in nc.m.functions:
        for bb in f.blocks:
            new_insts = []
            for inst in bb.instructions:
                si = inst.sync_info
                if si is not None and si.on_wait and len(si.on_wait) > max_waits:
                    waits = list(si.on_wait)
                    head, tail = waits[:-max_waits], waits[-max_waits:]
                    for i, w in enumerate(head):
                        nop = mybir.InstNoOp(
                            name=f"{inst.name}-ws{i}", engine=inst.engine,
                            ins=[], outs=[],
                        )
                        nop.sync_info = mybir.SyncInfo(on_wait=[w], on_update=[])
                        new_insts.append(nop)
                        n += 1
                    si.on_wait = tail
                new_insts.append(inst)
            bb.instructions[:] = new_insts
    return n


def build_nc(t_total=T, blk=16, legalize=True, dbg=False):
    """Build the single-core SPMD program. t_total timesteps, blk steps/block."""
    nblk = t_total // blk
    cpb = blk * BS            # cols per block
    ncols = t_total * BS      # total cols for this core
    nch = ncols // 128        # head row-chunks (2 steps each)

    nc = bass.Bass()
    if dbg:
        sm_o = nc.declare_dram_parameter("sm_o", [128, nch, NHD], F32, isOutput=True)
        meq_o = nc.declare_dram_parameter("meq_o", [128, nch, ACTN], F32, isOutput=True)

    # ---- dram parameters (per-core shards; transposed layouts host-side) ----
    xT = nc.declare_dram_parameter("xT", [OBS, ncols], F32, isOutput=False)
    done = nc.declare_dram_parameter("done", [ncols], F32, isOutput=False)
    actT = nc.declare_dram_parameter("actT", [128, nch], I32, isOutput=False)
    h0T = nc.declare_dram_parameter("h0T", [H, BS], F32, isOutput=False)
    c0T = nc.declare_dram_parameter("c0T", [H, BS], F32, isOutput=False)
    W1 = nc.declare_dram_parameter("W1", [OBS, H], F32, isOutput=False)
    b1 = nc.declare_dram_parameter("b1", [H], F32, isOutput=False)
    W2 = nc.declare_dram_parameter("W2", [H, H], F32, isOutput=False)
    b2 = nc.declare_dram_parameter("b2", [H], F32, isOutput=False)
    Wih = nc.declare_dram_parameter("Wih", [H, 4 * H], F32, isOutput=False)
    Whh = nc.declare_dram_parameter("Whh", [H, 4 * H], F32, isOutput=False)
    bih = nc.declare_dram_parameter("bih", [4 * H], F32, isOutput=False)
    bhh = nc.declare_dram_parameter("bhh", [4 * H], F32, isOutput=False)
    Whd = nc.declare_dram_parameter("Whd", [H, NHD], F32, isOutput=False)
    bhd = nc.declare_dram_parameter("bhd", [NHD], F32, isOutput=False)

    logp_o = nc.declare_dram_parameter("logp_o", [128, nch], F32, isOutput=True)
    ent_o = nc.declare_dram_parameter("ent_o", [128, nch], F32, isOutput=True)
    val_o = nc.declare_dram_parameter("val_o", [128, nch], F32, isOutput=True)
    hT_o = nc.declare_dram_parameter("hT_o", [KH, 128, BS], F32, isOutput=True)
    cT_o = nc.declare_dram_parameter("cT_o", [KH, 128, BS], F32, isOutput=True)

    from contextlib import ExitStack
    with tile.TileContext(nc) as tc, ExitStack() as es:
        wpool = es.enter_context(tc.tile_pool(name="weights", bufs=1))
        xp = es.enter_context(tc.tile_pool(name="xp", bufs=2))
        maskp = es.enter_context(tc.tile_pool(name="maskp", bufs=2))

        def load_inputs(b):
            """DMA x + mask for block b, compute m=1-done and xbf."""
            c0 = b * cpb
            lmask = cpb + BS if b < nblk - 1 else cpb
            mstg = maskp.tile([128, cpb + BS], F32, tag="mstg")
            nc.sync.dma_start(
                mstg[:, :lmask],
                done[c0:c0 + lmask].unsqueeze(0).partition_broadcast(128),
            )
            mb = maskp.tile([128, cpb + BS], F32, tag="mb")
            nc.vector.tensor_scalar(
                mb[:, :lmask], mstg[:, :lmask], -1.0, 1.0, ALU.mult, ALU.add
            )
            xs = xp.tile([128, cpb], F32, tag="xstg")
            nc.sync.dma_start(xs[:], xT[:, c0:c0 + cpb])
            xbf = xp.tile([128, cpb], BF16, tag="xbf")
            nc.vector.tensor_copy(xbf[:], xs[:])
            return mb, xbf

        blk0_in = load_inputs(0)

        with tc.tile_pool(name="stage", bufs=4) as stg:
            # ---- weights to SBUF, cast to bf16 ----
            def load_cast(dst_bf, src_ap, shape):
                s = stg.tile(shape, F32, tag="stg")
                nc.sync.dma_start(s[:], src_ap)
                nc.vector.tensor_copy(dst_bf, s[:])

            def load_cast_flat(dst_bf, src_flat, p, a, b):
                # flat stage + flat src -> big DMA descriptors; the cast
                # reshapes to the destination view
                s = stg.tile([p, a * b], F32, tag="stg")
                nc.sync.dma_start(s[:], src_flat)
                nc.vector.tensor_copy(dst_bf, s[:].rearrange("p (a b) -> p a b", a=a))

            W1sb = wpool.tile([128, KH, 128], BF16)   # [obs, m, c]
            load_cast(W1sb[:], W1[:].rearrange("p (m c) -> p m c", c=128), [128, KH, 128])
            W2sb = wpool.tile([128, KH, KH, 128], BF16)  # [p, k, m, c]
            for k in range(KH):
                load_cast_flat(
                    W2sb[:, k], W2[:].rearrange("(k p) g -> k p g", p=128)[k],
                    128, KH, 128,
                )
            Wihsb = wpool.tile([128, KH, MG, 128], BF16)
            Whhsb = wpool.tile([128, KH, MG, 128], BF16)
            for k in range(KH):
                load_cast_flat(
                    Wihsb[:, k], Wih[:].rearrange("(k p) g -> k p g", p=128)[k],
                    128, MG, 128,
                )
            for k in range(KH):
                load_cast_flat(
                    Whhsb[:, k], Whh[:].rearrange("(k p) g -> k p g", p=128)[k],
                    128, MG, 128,
                )
            Whdsb = wpool.tile([128, KH, NHD], BF16)
            load_cast(Whdsb[:], Whd[:].rearrange("(k p) a -> p k a", p=128), [128, KH, NHD])

            # biases stay fp32
            b1sb = wpool.tile([128, KH], F32)
            nc.sync.dma_start(b1sb[:], b1[:].rearrange("(k p) -> p k", p=128))
            b2sb = wpool.tile([128, KH], F32)
            nc.sync.dma_start(b2sb[:], b2[:].rearrange("(k p) -> p k", p=128))
            bg_i = stg.tile([128, MG], F32, tag="bg_i")
            nc.sync.dma_start(bg_i[:], bih[:].rearrange("(m p) -> p m", p=128))
            bg_h = stg.tile([128, MG], F32, tag="bg_h")
            nc.sync.dma_start(bg_h[:], bhh[:].rearrange("(m p) -> p m", p=128))
            bgsb = wpool.tile([128, MG], F32)
            nc.vector.tensor_add(bgsb[:], bg_i[:], bg_h[:])
            hbias_bc = wpool.tile([128, NHD], F32)
            nc.sync.dma_start(hbias_bc[:], bhd[:].unsqueeze(0).partition_broadcast(128))

            # identity for Xg psum-accumulate matmuls
            from concourse.masks import make_identity
            ident = wpool.tile([128, 128], BF16)
            make_identity(nc, ident[:])

            # iota over action dim + action values as fp32
            iota_i = stg.tile([128, ACTN], I32, tag="iota_i")
            nc.gpsimd.iota(iota_i[:], pattern=[[1, ACTN]], base=0, channel_multiplier=0)
            iota_f = wpool.tile([128, ACTN], F32)
            nc.vector.tensor_copy(iota_f[:], iota_i[:])
            act_i = stg.tile([128, nch], I32, tag="act_i")
            nc.sync.dma_start(act_i[:], actT[:])
            act_f = wpool.tile([128, nch], F32)
            nc.vector.tensor_copy(act_f[:], act_i[:])

            # h0/c0
            h0sb = wpool.tile([128, KH, BS], F32)
            nc.sync.dma_start(h0sb[:], h0T[:].rearrange("(k p) e -> p k e", p=128))
            c0sb = wpool.tile([128, KH, BS], F32)
            nc.sync.dma_start(c0sb[:], c0T[:].rearrange("(k p) e -> p k e", p=128))

        sm_all = wpool.tile([128, nch, NHD], F32)   # logits+value staging

        with (
            tc.tile_pool(name="actp", bufs=1) as actp,  # h1/ha single-buffered
            tc.tile_pool(name="xgp", bufs=2) as xgp,
            tc.tile_pool(name="state", bufs=2) as st,
            tc.tile_pool(name="hs2p", bufs=2) as hs2p,
            tc.tile_pool(name="mmps", bufs=4, space="PSUM") as mmps,
            tc.tile_pool(name="gifps", bufs=2, space="PSUM") as gifps,
            tc.tile_pool(name="ggps", bufs=1, space="PSUM") as ggps,
            tc.tile_pool(name="gops", bufs=1, space="PSUM") as gops,
        ):

            def make_bulk_units(xbf):
                """Return (xg_tile, [unit closures]) for one block's bulk work.
                Each unit emits one psum-group of matmuls + its epilogue."""
                h1 = actp.tile([128, KH, cpb], BF16, tag="h1")
                ha = actp.tile([128, KH, cpb], BF16, tag="ha")
                xg = xgp.tile([128, MG, cpb], BF16, tag="xg")
                units = []

                def u_mlp1(m, nn):
                    def go():
                        ps = mmps.tile([128, 512], F32, tag="mm")
                        nc.tensor.matmul(
                            ps[:], W1sb[:, m], xbf[:, nn * 512:(nn + 1) * 512],
                            start=True, stop=True,
                        )
                        nc.scalar.activation(
                            h1[:, m, nn * 512:(nn + 1) * 512], ps[:],
                            AF.Relu, bias=b1sb[:, m:m + 1],
                        )
                    return go

                def u_mlp2(m, nn):
                    def go():
                        ps = mmps.tile([128, 512], F32, tag="mm")
                        for k in range(KH):
                            nc.tensor.matmul(
                                ps[:], W2sb[:, k, m], h1[:, k, nn * 512:(nn + 1) * 512],
                                start=(k == 0), stop=(k == KH - 1),
                            )
                        nc.scalar.activation(
                            ha[:, m, nn * 512:(nn + 1) * 512], ps[:],
                            AF.Relu, bias=b2sb[:, m:m + 1],
                        )
                    return go

                def u_xg(m, nn):
                    def go():
                        ps = mmps.tile([128, 512], F32, tag="mm")
                        for k in range(KH):
                            nc.tensor.matmul(
                                ps[:], Wihsb[:, k, m], ha[:, k, nn * 512:(nn + 1) * 512],
                                start=(k == 0), stop=(k == KH - 1),
                            )
                        nc.scalar.activation(
                            xg[:, m, nn * 512:(nn + 1) * 512], ps[:],
                            AF.Identity, bias=bgsb[:, m:m + 1],
                        )
                    return go

                for m in range(KH):
                    for nn in range(cpb // 512):
                        units.append(u_mlp1(m, nn))
                for m in range(KH):
                    for nn in range(cpb // 512):
                        units.append(u_mlp2(m, nn))
                for m in range(MG):
                    for nn in range(cpb // 512):
                        units.append(u_xg(m, nn))
                return xg, units

            # ---- prologue: block 0 inputs + bulk up front ----
            mb, xbf0 = blk0_in
            xg, units0 = make_bulk_units(xbf0)
            for u in units0:
                u()

            # ---- init state (uses mb[0]) ----
            hm = st.tile([128, KH, BS], BF16, tag="hm")
            m0 = mb[:, 0:BS].unsqueeze(1).broadcast_to([128, KH, BS])
            nc.vector.tensor_mul(hm[:], h0sb[:], m0)
            cm = st.tile([128, KH, BS], F32, tag="cm")
            nc.vector.tensor_mul(cm[:], c0sb[:], m0)

            hs2 = None
            for b in range(nblk):
                if b + 1 < nblk:
                    mb_next, xbf_next = load_inputs(b + 1)
                    xg_next, units = make_bulk_units(xbf_next)
                else:
                    mb_next, xg_next, units = None, None, []
                upos = 0

                for s in range(blk):
                    t = b * blk + s
                    # gate matmuls into 3 bank-granular psum tiles so the
                    # adds can start as soon as each gate group finishes
                    g_if = gifps.tile([128, 8, BS], F32, tag="gif")
                    g_g = ggps.tile([128, KH, BS], F32, tag="gg")
                    g_o = gops.tile([128, KH, BS], F32, tag="go")
                    xgs = xg[:, :, s * BS:(s + 1) * BS]
                    # psum groups open with the X-part (identity matmul), the
                    # gate matmuls accumulate on top; activations read psum
                    def gate_group(gt, xslice, m0, m1):
                        nc.tensor.matmul(gt[:], ident[:], xslice,
                                         start=True, stop=False)
                        for m in range(m0, m1):
                            for k in range(KH):
                                nc.tensor.matmul(
                                    gt[:, m - m0], Whhsb[:, k, m], hm[:, k],
                                    start=False,
                                    stop=(m == m1 - 1 and k == KH - 1),
                                )
                    gate_group(g_if, xgs[:, 0:8], 0, 8)
                    sig_if = st.tile([128, 8, BS], F32, tag="sig_if")
                    nc.scalar.activation(sig_if[:], g_if[:], AF.Sigmoid)
                    gate_group(g_g, xgs[:, 8:12], 8, 12)
                    tanh_g = st.tile([128, KH, BS], F32, tag="tanh_g")
                    nc.scalar.activation(tanh_g[:], g_g[:], AF.Tanh)
                    t1 = st.tile([128, KH, BS], F32, tag="t1")
                    nc.vector.tensor_mul(t1[:], sig_if[:, 4:8], cm[:])
                    gate_group(g_o, xgs[:, 12:16], 12, 16)
                    sig_o = st.tile([128, KH, BS], F32, tag="sig_o")
                    nc.scalar.activation(sig_o[:], g_o[:], AF.Sigmoid)
                    t2 = st.tile([128, KH, BS], F32, tag="t2")
                    nc.vector.tensor_mul(t2[:], sig_if[:, 0:4], tanh_g[:])
                    cnew = st.tile([128, KH, BS], F32, tag="cnew")
                    nc.vector.tensor_add(cnew[:], t1[:], t2[:])
                    tanh_c = st.tile([128, KH, BS], F32, tag="tanh_c")
                    nc.scalar.activation(tanh_c[:], cnew[:], AF.Tanh)

                    if s % 2 == 0:
                        hs2 = hs2p.tile([128, KH, 128], BF16, tag="hs2")
                    hslice = hs2[:, :, (s % 2) * BS:(s % 2) * BS + BS]

                    if t < t_total - 1:
                        mnext = (
                            mb[:, (s + 1) * BS:(s + 2) * BS]
                            .unsqueeze(1).broadcast_to([128, KH, BS])
                        )
                        # masked sig_o first: hm = (sig_o*m) * tanh_c shortens
                        # the critical path to the next step's matmuls
                        som = st.tile([128, KH, BS], F32, tag="som")
                        nc.vector.tensor_mul(som[:], sig_o[:], mnext)
                        hm = st.tile([128, KH, BS], BF16, tag="hm")
                        nc.vector.tensor_mul(hm[:], som[:], tanh_c[:])
                        nc.vector.tensor_mul(hslice, sig_o[:], tanh_c[:])
                        cm = st.tile([128, KH, BS], F32, tag="cm")
                        nc.vector.tensor_mul(cm[:], cnew[:], mnext)
                    else:
                        nc.vector.tensor_mul(hslice, sig_o[:], tanh_c[:])
                        hT_f = st.tile([128, KH, BS], F32, tag="hT_f")
                        nc.vector.tensor_mul(hT_f[:], sig_o[:], tanh_c[:])
                        nc.sync.dma_start(hT_o[:].transpose([1, 0, 2]), hT_f[:])
                        nc.sync.dma_start(cT_o[:].transpose([1, 0, 2]), cnew[:])

                    # head matmuls every 2 steps
                    if s % 2 == 1:
                        rc = t // 2
                        hp = mmps.tile([128, NHD], F32, tag="mm")
                        for k in range(KH):
                            nc.tensor.matmul(
                                hp[:], hs2[:, k], Whdsb[:, k],
                                start=(k == 0), stop=(k == KH - 1),
                            )
                        nc.vector.tensor_copy(sm_all[:, rc], hp[:])

                    # interleave next block's bulk work into this step's gaps
                    ulim = ((s + 1) * len(units) + blk - 1) // blk
                    while upos < ulim:
                        units[upos]()
                        upos += 1

                mb, xg = mb_next, xg_next

            # ---------- final phase: batched softmax/entropy/gather ----------
            with tc.tile_pool(name="fin", bufs=1) as fin:
                sm2 = fin.tile([128, nch, NHD], F32)
                nc.vector.tensor_add(
                    sm2[:],
                    sm_all[:],
                    hbias_bc[:].unsqueeze(1).broadcast_to([128, nch, NHD]),
                )
                logits = sm2[:, :, 0:ACTN]
                et = fin.tile([128, nch, ACTN], F32)
                nc.scalar.activation(et[:], logits, AF.Exp)
                ssum = fin.tile([128, nch], F32)
                nc.vector.reduce_sum(ssum[:].unsqueeze(2), et[:], axis=mybir.AxisListType.X)
                logs = fin.tile([128, nch], F32)
                nc.scalar.activation(logs[:], ssum[:], AF.Ln)
                lpa = fin.tile([128, nch, ACTN], F32)
                nc.vector.tensor_sub(
                    lpa[:], logits, logs[:].unsqueeze(2).broadcast_to([128, nch, ACTN])
                )
                # entropy = -(sum e*lpa)/ssum
                tt = fin.tile([128, nch, ACTN], F32)
                nc.vector.tensor_mul(tt[:], et[:], lpa[:])
                es = fin.tile([128, nch], F32)
                nc.vector.reduce_sum(es[:].unsqueeze(2), tt[:], axis=mybir.AxisListType.X)
                rec = fin.tile([128, nch], F32)
                nc.vector.reciprocal(rec[:], ssum[:])
                env = fin.tile([128, nch], F32)
                nc.vector.tensor_mul(env[:], es[:], rec[:])
                ent_t = fin.tile([128, nch], F32)
                nc.vector.tensor_scalar_mul(ent_t[:], env[:], -1.0)
                nc.sync.dma_start(ent_o[:], ent_t[:])
                # logp = sum(lpa * (iota == action))
                meq = fin.tile([128, nch, ACTN], F32, tag="tt")
                nc.vector.tensor_tensor(
                    meq[:],
                    iota_f[:].unsqueeze(1).broadcast_to([128, nch, ACTN]),
                    act_f[:].unsqueeze(2).broadcast_to([128, nch, ACTN]),
                    ALU.is_equal,
                )
                if dbg:
                    nc.sync.dma_start(sm_o[:], sm_all[:])
                    nc.sync.dma_start(meq_o[:], meq[:])
                lpm = fin.tile([128, nch, ACTN], F32, tag="et")
                nc.vector.tensor_mul(lpm[:], lpa[:], meq[:])
                lp_t = fin.tile([128, nch], F32)
                nc.vector.reduce_sum(lp_t[:].unsqueeze(2), lpm[:], axis=mybir.AxisListType.X)
                nc.sync.dma_start(logp_o[:], lp_t[:])
                val_t = fin.tile([128, nch], F32)
                nc.vector.tensor_copy(
                    val_t[:].unsqueeze(2), sm2[:, :, ACTN:ACTN + 1]
                )
                nc.sync.dma_start(val_o[:], val_t[:])

    if legalize:
        split_wide_waits(nc)
    return nc


_NC_CACHE = {}


def _get_nc(t_total=T, blk=16):
    key = (t_total, blk)
    if key not in _NC_CACHE:
        _NC_CACHE[key] = build_nc(t_total, blk)
    return _NC_CACHE[key]


def make_in_maps(x, done, action, h0a, c0a, h0c, c0c,
                 W1a, b1a, W2a, b2a, Wih_a, Whh_a, bih_a, bhh_a, Wact, bact,
                 W1c, b1c, W2c, b2c, Wih_c, Whh_c, bih_c, bhh_c, Wcrit, bcrit,
                 t_total=T):
    n = t_total * B
    x = np.asarray(x, np.float32).reshape(t_total, B, OBS)
    done = np.asarray(done, np.float32).reshape(t_total, B)
    action = np.asarray(action, np.int32).reshape(t_total, B)
    c = np.ascontiguousarray

    whd = np.concatenate([np.asarray(Wact), np.asarray(Wcrit)], axis=1)
    bhd = np.concatenate([np.asarray(bact), np.asarray(bcrit)])
    branch = {
        "a": dict(W1=W1a, b1=b1a, W2=W2a, b2=b2a, Wih=Wih_a, Whh=Whh_a,
                  bih=bih_a, bhh=bhh_a, Whd=whd, bhd=bhd, h0=h0a, c0=c0a),
        "c": dict(W1=W1c, b1=b1c, W2=W2c, b2=b2c, Wih=Wih_c, Whh=Whh_c,
                  bih=bih_c, bhh=bhh_c, Whd=whd, bhd=bhd, h0=h0c, c0=c0c),
    }
    in_maps = []
    for core in range(NCORES):
        br = branch["a" if core < 4 else "c"]
        e0 = 64 * (core % 4)
        xs = x[:, e0:e0 + BS].reshape(t_total * BS, OBS)
        ds = done[:, e0:e0 + BS].reshape(t_total * BS)
        As = action[:, e0:e0 + BS].reshape(t_total * BS)
        nch = (t_total * BS) // 128
        in_maps.append({
            "xT": c(xs.T.astype(np.float32)),
            "done": c(ds.astype(np.float32)),
            "actT": c(As.reshape(nch, 128).T.astype(np.int32)),
            "h0T": c(np.asarray(br["h0"], np.float32)[e0:e0 + BS].T),
            "c0T": c(np.asarray(br["c0"], np.float32)[e0:e0 + BS].T),
            "W1": c(np.asarray(br["W1"], np.float32)),
            "b1": c(np.asarray(br["b1"], np.float32)),
            "W2": c(np.asarray(br["W2"], np.float32)),
            "b2": c(np.asarray(br["b2"], np.float32)),
            "Wih": c(np.asarray(br["Wih"], np.float32)),
            "Whh": c(np.asarray(br["Whh"], np.float32)),
            "bih": c(np.asarray(br["bih"], np.float32)),
            "bhh": c(np.asarray(br["bhh"], np.float32)),
            "Whd": c(np.asarray(br["Whd"], np.float32)),
            "bhd": c(np.asarray(br["bhd"], np.float32)),
        })
    return in_maps


def assemble(results, t_total=T):
    n = t_total * B
    logp = np.zeros((t_total, B), np.float32)
    entropy = np.zeros((t_total, B), np.float32)
    value = np.zeros((t_total, B), np.float32)
    hTa = np.zeros((B, H), np.float32)
    cTa = np.zeros((B, H), np.float32)
    hTc = np.zeros((B, H), np.float32)
    cTc = np.zeros((B, H), np.float32)
    for core in range(NCORES):
        r = results[core]
        e0 = 64 * (core % 4)
        hT = r["hT_o"].reshape(H, BS).T   # [BS, H]
        cT = r["cT_o"].reshape(H, BS).T
        if core < 4:
            logp[:, e0:e0 + BS] = r["logp_o"].T.reshape(t_total, BS)
            entropy[:, e0:e0 + BS] = r["ent_o"].T.reshape(t_total, BS)
            hTa[e0:e0 + BS] = hT
            cTa[e0:e0 + BS] = cT
        else:
            value[:, e0:e0 + BS] = r["val_o"].T.reshape(t_total, BS)
            hTc[e0:e0 + BS] = hT
            cTc[e0:e0 + BS] = cT
    return (logp.reshape(n), entropy.reshape(n), value.reshape(n, 1),
            hTa, cTa, hTc, cTc)


def kernel(**inputs):
    nc = _get_nc()
    in_maps = make_in_maps(**inputs)
    res = run_bass_kernel_spmd(nc, in_maps, list(range(NCORES)))
    return assemble(res.results)


# revision 44
# speedup vs baseline: 1.0071x; 1.0071x over previous
"""Trainium2 Bass kernel for the actor-critic LSTM agent (nn_Agent_37984690765937).

Strategy:
  - 8 cores, 4+4 split: cores 0-3 run the ACTOR branch, cores 4-7 the CRITIC
    branch; each core owns a block of 64 envs (B=256 = 4 blocks x 64).
    Both branches run the identical SPMD program; only the weights/inputs
    fed to each core differ.  Actor cores' value output is discarded by the
    host, critic cores' logp/entropy likewise.
  - Everything on-chip is kept in TRANSPOSED (feature-major) layout so that
    activations are always matmul-ready ([K-chunks x 128, cols]); the moving
    operand of every matmul is streamed with the batch/cols on the free dim.
  - Per-core pipeline over 8 blocks of 16 timesteps (1024 cols):
      MLP1 -> MLP2 -> X-gates precompute (bf16 matmuls, fp32 psum)
      -> 16 masked-LSTM steps (h stationary bf16, Whh moving bf16, c fp32)
      -> per-2-step head matmuls (logits+value fused [*,19])
    Tile's scheduler overlaps block b+1's bulk matmuls with block b's LSTM.
  - Softmax / log / entropy / action-gather are deferred to one batched
    final phase (exp/ln live in a different ACT table set than sigmoid/tanh,
    so interleaving would thrash the activation tables).
"""

import sys

if "/opt/trn_rl_repo" not in sys.path:
    sys.path.insert(0, "/opt/trn_rl_repo")

import numpy as np

import concourse.bass as bass
import concourse.tile as tile
from concourse import mybir
from concourse.bass_utils import run_bass_kernel_spmd

AF = mybir.ActivationFunctionType
ALU = mybir.AluOpType
F32 = mybir.dt.float32
BF16 = mybir.dt.bfloat16
I32 = mybir.dt.int32

OBS, ACTN, H, T, B = 128, 18, 512, 128, 256
NCORES = 8
BS = 64          # envs per core
KO = OBS // 128  # 1
KH = H // 128    # 4
MG = (4 * H) // 128  # 16 gate chunks
NHD = ACTN + 1   # fused logits+value head width


def split_wide_waits(nc, max_waits=1):
    """This walrus build supports only one sync-wait per instruction; move
    extra waits onto preceding single-wait NoOps on the same engine."""
    n = 0
    for f in nc.m.functions:
        for bb in f.blocks:
            new_insts = []
            for inst in bb.instructions:
                si = inst.sync_info
                if si is not None and si.on_wait and len(si.on_wait) > max_waits:
                    waits = list(si.on_wait)
                    head, tail = waits[:-max_waits], waits[-max_waits:]
                    for i, w in enumerate(head):
                        nop = mybir.InstNoOp(
                            name=f"{inst.name}-ws{i}", engine=inst.engine,
                            ins=[], outs=[],
                        )
                        nop.sync_info = mybir.SyncInfo(on_wait=[w], on_update=[])
                        new_insts.append(nop)
                        n += 1
                    si.on_wait = tail
                new_insts.append(inst)
            bb.instructions[:] = new_insts
    return n


def build_nc(t_total=T, blk=16, legalize=True, dbg=False):
    """Build the single-core SPMD program. t_total timesteps, blk steps/block."""
    nblk = t_total // blk
    cpb = blk * BS            # cols per block
    ncols = t_total * BS      # total cols for this core
    nch = ncols // 128        # head row-chunks (2 steps each)

    nc = bass.Bass()
    if dbg:
        sm_o = nc.declare_dram_parameter("sm_o", [128, nch, NHD], F32, isOutput=True)
        meq_o = nc.declare_dram_parameter("meq_o", [128, nch, ACTN], F32, isOutput=True)

    # ---- dram parameters (per-core shards; transposed layouts host-side) ----
    xT = nc.declare_dram_parameter("xT", [OBS, ncols], F32, isOutput=False)
    done = nc.declare_dram_parameter("done", [ncols], F32, isOutput=False)
    actT = nc.declare_dram_parameter("actT", [128, nch], I32, isOutput=False)
    h0T = nc.declare_dram_parameter("h0T", [H, BS], F32, isOutput=False)
    c0T = nc.declare_dram_parameter("c0T", [H, BS], F32, isOutput=False)
    W1 = nc.declare_dram_parameter("W1", [OBS, H], F32, isOutput=False)
    b1 = nc.declare_dram_parameter("b1", [H], F32, isOutput=False)
    W2 = nc.declare_dram_parameter("W2", [H, H], F32, isOutput=False)
    b2 = nc.declare_dram_parameter("b2", [H], F32, isOutput=False)
    Wih = nc.declare_dram_parameter("Wih", [H, 4 * H], F32, isOutput=False)
    Whh = nc.declare_dram_parameter("Whh", [H, 4 * H], F32, isOutput=False)
    bih = nc.declare_dram_parameter("bih", [4 * H], F32, isOutput=False)
    bhh = nc.declare_dram_parameter("bhh", [4 * H], F32, isOutput=False)
    Whd = nc.declare_dram_parameter("Whd", [H, NHD], F32, isOutput=False)
    bhd = nc.declare_dram_parameter("bhd", [NHD], F32, isOutput=False)

    logp_o = nc.declare_dram_parameter("logp_o", [128, nch], F32, isOutput=True)
    ent_o = nc.declare_dram_parameter("ent_o", [128, nch], F32, isOutput=True)
    val_o = nc.declare_dram_parameter("val_o", [128, nch], F32, isOutput=True)
    hT_o = nc.declare_dram_parameter("hT_o", [KH, 128, BS], F32, isOutput=True)
    cT_o = nc.declare_dram_parameter("cT_o", [KH, 128, BS], F32, isOutput=True)

    from contextlib import ExitStack
    with tile.TileContext(nc) as tc, ExitStack() as es:
        wpool = es.enter_context(tc.tile_pool(name="weights", bufs=1))
        xp = es.enter_context(tc.tile_pool(name="xp", bufs=2))
        maskp = es.enter_context(tc.tile_pool(name="maskp", bufs=2))

        def load_inputs(b):
            """DMA x + mask for block b, compute m=1-done and xbf."""
            c0 = b * cpb
            lmask = cpb + BS if b < nblk - 1 else cpb
            mstg = maskp.tile([128, cpb + BS], F32, tag="mstg")
            nc.sync.dma_start(
                mstg[:, :lmask],
                done[c0:c0 + lmask].unsqueeze(0).partition_broadcast(128),
            )
            mb = maskp.tile([128, cpb + BS], F32, tag="mb")
            nc.vector.tensor_scalar(
                mb[:, :lmask], mstg[:, :lmask], -1.0, 1.0, ALU.mult, ALU.add
            )
            xs = xp.tile([128, cpb], F32, tag="xstg")
            nc.sync.dma_start(xs[:], xT[:, c0:c0 + cpb])
            xbf = xp.tile([128, cpb], BF16, tag="xbf")
            nc.vector.tensor_copy(xbf[:], xs[:])
            return mb, xbf

        blk0_in = load_inputs(0)

        with tc.tile_pool(name="stage", bufs=4) as stg:
            # ---- weights to SBUF, cast to bf16 ----
            def load_cast(dst_bf, src_ap, shape):
                s = stg.tile(shape, F32, tag="stg")
                nc.sync.dma_start(s[:], src_ap)
                nc.vector.tensor_copy(dst_bf, s[:])

            def load_cast_flat(dst_bf, src_flat, p, a, b):
                # flat stage + flat src -> big DMA descriptors; the cast
                # reshapes to the destination view
                s = stg.tile([p, a * b], F32, tag="stg")
                nc.sync.dma_start(s[:], src_flat)
                nc.vector.tensor_copy(dst_bf, s[:].rearrange("p (a b) -> p a b", a=a))

            W1sb = wpool.tile([128, KH, 128], BF16)   # [obs, m, c]
            load_cast(W1sb[:], W1[:].rearrange("p (m c) -> p m c", c=128), [128, KH, 128])
            W2sb = wpool.tile([128, KH, KH, 128], BF16)  # [p, k, m, c]
            for k in range(KH):
                load_cast_flat(
                    W2sb[:, k], W2[:].rearrange("(k p) g -> k p g", p=128)[k],
                    128, KH, 128,
                )
            Wihsb = wpool.tile([128, KH, MG, 128], BF16)
            Whhsb = wpool.tile([128, KH, MG, 128], BF16)
            for k in range(KH):
                load_cast_flat(
                    Wihsb[:, k], Wih[:].rearrange("(k p) g -> k p g", p=128)[k],
                    128, MG, 128,
                )
                load_cast_flat(
                    Whhsb[:, k], Whh[:].rearrange("(k p) g -> k p g", p=128)[k],
                    128, MG, 128,
                )
            Whdsb = wpool.tile([128, KH, NHD], BF16)
            load_cast(Whdsb[:], Whd[:].rearrange("(k p) a -> p k a", p=128), [128, KH, NHD])

            # biases stay fp32
            b1sb = wpool.tile([128, KH], F32)
            nc.sync.dma_start(b1sb[:], b1[:].rearrange("(k p) -> p k", p=128))
            b2sb = wpool.tile([128, KH], F32)
            nc.sync.dma_start(b2sb[:], b2[:].rearrange("(k p) -> p k", p=128))
            bg_i = stg.tile([128, MG], F32, tag="bg_i")
            nc.sync.dma_start(bg_i[:], bih[:].rearrange("(m p) -> p m", p=128))
            bg_h = stg.tile([128, MG], F32, tag="bg_h")
            nc.sync.dma_start(bg_h[:], bhh[:].rearrange("(m p) -> p m", p=128))
            bgsb = wpool.tile([128, MG], F32)
            nc.vector.tensor_add(bgsb[:], bg_i[:], bg_h[:])
            hbias_bc = wpool.tile([128, NHD], F32)
            nc.sync.dma_start(hbias_bc[:], bhd[:].unsqueeze(0).partition_broadcast(128))

            # identity for Xg psum-accumulate matmuls
            from concourse.masks import make_identity
            ident = wpool.tile([128, 128], BF16)
            make_identity(nc, ident[:])

            # iota over action dim + action values as fp32
            iota_i = stg.tile([128, ACTN], I32, tag="iota_i")
            nc.gpsimd.iota(iota_i[:], pattern=[[1, ACTN]], base=0, channel_multiplier=0)
            iota_f = wpool.tile([128, ACTN], F32)
            nc.vector.tensor_copy(iota_f[:], iota_i[:])
            act_i = stg.tile([128, nch], I32, tag="act_i")
            nc.sync.dma_start(act_i[:], actT[:])
            act_f = wpool.tile([128, nch], F32)
            nc.vector.tensor_copy(act_f[:], act_i[:])

            # h0/c0
            h0sb = wpool.tile([128, KH, BS], F32)
            nc.sync.dma_start(h0sb[:], h0T[:].rearrange("(k p) e -> p k e", p=128))
            c0sb = wpool.tile([128, KH, BS], F32)
            nc.sync.dma_start(c0sb[:], c0T[:].rearrange("(k p) e -> p k e", p=128))

        sm_all = wpool.tile([128, nch, NHD], F32)   # logits+value staging

        with (
            tc.tile_pool(name="actp", bufs=1) as actp,  # h1/ha single-buffered
            tc.tile_pool(name="xgp", bufs=2) as xgp,
            tc.tile_pool(name="state", bufs=2) as st,
            tc.tile_pool(name="hs2p", bufs=2) as hs2p,
            tc.tile_pool(name="mmps", bufs=4, space="PSUM") as mmps,
            tc.tile_pool(name="gifps", bufs=2, space="PSUM") as gifps,
            tc.tile_pool(name="ggps", bufs=1, space="PSUM") as ggps,
            tc.tile_pool(name="gops", bufs=1, space="PSUM") as gops,
        ):

            def make_bulk_units(xbf):
                """Return (xg_tile, [unit closures]) for one block's bulk work.
                Each unit emits one psum-group of matmuls + its epilogue."""
                h1 = actp.tile([128, KH, cpb], BF16, tag="h1")
                ha = actp.tile([128, KH, cpb], BF16, tag="ha")
                xg = xgp.tile([128, MG, cpb], BF16, tag="xg")
                units = []

                def u_mlp1(m, nn):
                    def go():
                        ps = mmps.tile([128, 512], F32, tag="mm")
                        nc.tensor.matmul(
                            ps[:], W1sb[:, m], xbf[:, nn * 512:(nn + 1) * 512],
                            start=True, stop=True,
                        )
                        nc.scalar.activation(
                            h1[:, m, nn * 512:(nn + 1) * 512], ps[:],
                            AF.Relu, bias=b1sb[:, m:m + 1],
                        )
                    return go

                def u_mlp2(m, nn):
                    def go():
                        ps = mmps.tile([128, 512], F32, tag="mm")
                        for k in range(KH):
                            nc.tensor.matmul(
                                ps[:], W2sb[:, k, m], h1[:, k, nn * 512:(nn + 1) * 512],
                                start=(k == 0), stop=(k == KH - 1),
                            )
                        nc.scalar.activation(
                            ha[:, m, nn * 512:(nn + 1) * 512], ps[:],
                            AF.Relu, bias=b2sb[:, m:m + 1],
                        )
                    return go

                def u_xg(m, nn):
                    def go():
                        ps = mmps.tile([128, 512], F32, tag="mm")
                        for k in range(KH):
                            nc.tensor.matmul(
                                ps[:], Wihsb[:, k, m], ha[:, k, nn * 512:(nn + 1) * 512],
                                start=(k == 0), stop=(k == KH - 1),
                            )
                        nc.scalar.activation(
                            xg[:, m, nn * 512:(nn + 1) * 512], ps[:],
                            AF.Identity, bias=bgsb[:, m:m + 1],
                        )
                    return go

                for m in range(KH):
                    for nn in range(cpb // 512):
                        units.append(u_mlp1(m, nn))
                for m in range(KH):
                    for nn in range(cpb // 512):
                        units.append(u_mlp2(m, nn))
                for m in range(MG):
                    for nn in range(cpb // 512):
                        units.append(u_xg(m, nn))
                return xg, units

            # ---- prologue: block 0 inputs + bulk up front ----
            mb, xbf0 = blk0_in
            xg, units0 = make_bulk_units(xbf0)
            for u in units0:
                u()

            # ---- init state (uses mb[0]) ----
            hm = st.tile([128, KH, BS], BF16, tag="hm")
            m0 = mb[:, 0:BS].unsqueeze(1).broadcast_to([128, KH, BS])
            nc.vector.tensor_mul(hm[:], h0sb[:], m0)
            cm = st.tile([128, KH, BS], F32, tag="cm")
            nc.vector.tensor_mul(cm[:], c0sb[:], m0)

            hs2 = None
            for b in range(nblk):
                if b + 1 < nblk:
                    mb_next, xbf_next = load_inputs(b + 1)
                    xg_next, units = make_bulk_units(xbf_next)
                else:
                    mb_next, xg_next, units = None, None, []
                upos = 0

                for s in range(blk):
                    t = b * blk + s
                    # gate matmuls into 3 bank-granular psum tiles so the
                    # adds can start as soon as each gate group finishes
                    g_if = gifps.tile([128, 8, BS], F32, tag="gif")
                    g_g = ggps.tile([128, KH, BS], F32, tag="gg")
                    g_o = gops.tile([128, KH, BS], F32, tag="go")
                    xgs = xg[:, :, s * BS:(s + 1) * BS]
                    # psum groups open with the X-part (identity matmul), the
                    # gate matmuls accumulate on top; activations read psum
                    def gate_group(gt, xslice, m0, m1):
                        nc.tensor.matmul(gt[:], ident[:], xslice,
                                         start=True, stop=False)
                        for m in range(m0, m1):
                            for k in range(KH):
                                nc.tensor.matmul(
                                    gt[:, m - m0], Whhsb[:, k, m], hm[:, k],
                                    start=False,
                                    stop=(m == m1 - 1 and k == KH - 1),
                                )
                    gate_group(g_if, xgs[:, 0:8], 0, 8)
                    sig_if = st.tile([128, 8, BS], F32, tag="sig_if")
                    nc.scalar.activation(sig_if[:], g_if[:], AF.Sigmoid)
                    gate_group(g_g, xgs[:, 8:12], 8, 12)
                    tanh_g = st.tile([128, KH, BS], F32, tag="tanh_g")
                    nc.scalar.activation(tanh_g[:], g_g[:], AF.Tanh)
                    t1 = st.tile([128, KH, BS], F32, tag="t1")
                    nc.vector.tensor_mul(t1[:], sig_if[:, 4:8], cm[:])
                    gate_group(g_o, xgs[:, 12:16], 12, 16)
                    sig_o = st.tile([128, KH, BS], F32, tag="sig_o")
                    nc.scalar.activation(sig_o[:], g_o[:], AF.Sigmoid)
                    t2 = st.tile([128, KH, BS], F32, tag="t2")
                    nc.vector.tensor_mul(t2[:], sig_if[:, 0:4], tanh_g[:])
                    cnew = st.tile([128, KH, BS], F32, tag="cnew")
                    nc.vector.tensor_add(cnew[:], t1[:], t2[:])
                    tanh_c = st.tile([128, KH, BS], F32, tag="tanh_c")
                    nc.scalar.activation(tanh_c[:], cnew[:], AF.Tanh)

                    if s % 2 == 0:
                        hs2 = hs2p.tile([128, KH, 128], BF16, tag="hs2")
                    hslice = hs2[:, :, (s % 2) * BS:(s % 2) * BS + BS]

                    if t < t_total - 1:
                        mnext = (
                            mb[:, (s + 1) * BS:(s + 2) * BS]
                            .unsqueeze(1).broadcast_to([128, KH, BS])
                        )
                        # masked sig_o first: hm = (sig_o*m) * tanh_c shortens
                        # the critical path to the next step's matmuls
                        som = st.tile([128, KH, BS], F32, tag="som")
                        nc.vector.tensor_mul(som[:], sig_o[:], mnext)
                        hm = st.tile([128, KH, BS], BF16, tag="hm")
                        nc.vector.tensor_mul(hm[:], som[:], tanh_c[:])
                        nc.vector.tensor_mul(hslice, sig_o[:], tanh_c[:])
                        cm = st.tile([128, KH, BS], F32, tag="cm")
                        nc.vector.tensor_mul(cm[:], cnew[:], mnext)
                    else:
                        nc.vector.tensor_mul(hslice, sig_o[:], tanh_c[:])
                        hT_f = st.tile([128, KH, BS], F32, tag="hT_f")
                        nc.vector.tensor_mul(hT_f[:], sig_o[:], tanh_c[:])
                        nc.sync.dma_start(hT_o[:].transpose([1, 0, 2]), hT_f[:])
                        nc.sync.dma_start(cT_o[:].transpose([1, 0, 2]), cnew[:])

                    # head matmuls every 2 steps
                    if s % 2 == 1:
                        rc = t // 2
                        hp = mmps.tile([128, NHD], F32, tag="mm")
                        for k in range(KH):
                            nc.tensor.matmul(
                                hp[:], hs2[:, k], Whdsb[:, k],
                                start=(k == 0), stop=(k == KH - 1),
                            )
                        nc.vector.tensor_copy(sm_all[:, rc], hp[:])

                    # interleave next block's bulk work into this step's gaps
                    ulim = ((s + 1) * len(units) + blk - 1) // blk
                    while upos < ulim:
                        units[upos]()
                        upos += 1

                mb, xg = mb_next, xg_next

            # ---------- final phase: batched softmax/entropy/gather ----------
            with tc.tile_pool(name="fin", bufs=1) as fin:
                sm2 = fin.tile([128, nch, NHD], F32)
                nc.vector.tensor_add(
                    sm2[:],
                    sm_all[:],
                    hbias_bc[:].unsqueeze(1).broadcast_to([128, nch, NHD]),
                )
                logits = sm2[:, :, 0:ACTN]
                et = fin.tile([128, nch, ACTN], F32)
                nc.scalar.activation(et[:], logits, AF.Exp)
                ssum = fin.tile([128, nch], F32)
                nc.vector.reduce_sum(ssum[:].unsqueeze(2), et[:], axis=mybir.AxisListType.X)
                logs = fin.tile([128, nch], F32)
                nc.scalar.activation(logs[:], ssum[:], AF.Ln)
                lpa = fin.tile([128, nch, ACTN], F32)
                nc.vector.tensor_sub(
                    lpa[:], logits, logs[:].unsqueeze(2).broadcast_to([128, nch, ACTN])
                )
                # entropy = -(sum e*lpa)/ssum
                tt = fin.tile([128, nch, ACTN], F32)
                nc.vector.tensor_mul(tt[:], et[:], lpa[:])
                es = fin.tile([128, nch], F32)
                nc.vector.reduce_sum(es[:].unsqueeze(2), tt[:], axis=mybir.AxisListType.X)
                rec = fin.tile([128, nch], F32)
                nc.vector.reciprocal(rec[:], ssum[:])
                env = fin.tile([128, nch], F32)
                nc.vector.tensor_mul(env[:], es[:], rec[:])
                ent_t = fin.tile([128, nch], F32)
                nc.vector.tensor_scalar_mul(ent_t[:], env[:], -1.0)
                nc.sync.dma_start(ent_o[:], ent_t[:])
                # logp = sum(lpa * (iota == action))
                meq = fin.tile([128, nch, ACTN], F32, tag="tt")
                nc.vector.tensor_tensor(
                    meq[:],
                    iota_f[:].unsqueeze(1).broadcast_to([128, nch, ACTN]),
                    act_f[:].unsqueeze(2).broadcast_to([128, nch, ACTN]),
                    ALU.is_equal,
                )
                if dbg:
                    nc.sync.dma_start(sm_o[:], sm_all[:])
                    nc.sync.dma_start(meq_o[:], meq[:])
                lpm = fin.tile([128, nch, ACTN], F32, tag="et")
                nc.vector.tensor_mul(lpm[:], lpa[:], meq[:])
                lp_t = fin.tile([128, nch], F32)
                nc.vector.reduce_sum(lp_t[:].unsqueeze(2), lpm[:], axis=mybir.AxisListType.X)
                nc.sync.dma_start(logp_o[:], lp_t[:])
                val_t = fin.tile([128, nch], F32)
                nc.vector.tensor_copy(
                    val_t[:].unsqueeze(2), sm2[:, :, ACTN:ACTN + 1]
                )
                nc.sync.dma_start(val_o[:], val_t[:])

    if legalize:
        split_wide_waits(nc)
    return nc


_NC_CACHE = {}


def _get_nc(t_total=T, blk=16):
    key = (t_total, blk)
    if key not in _NC_CACHE:
        _NC_CACHE[key] = build_nc(t_total, blk)
    return _NC_CACHE[key]


def make_in_maps(x, done, action, h0a, c0a, h0c, c0c,
                 W1a, b1a, W2a, b2a, Wih_a, Whh_a, bih_a, bhh_a, Wact, bact,
                 W1c, b1c, W2c, b2c, Wih_c, Whh_c, bih_c, bhh_c, Wcrit, bcrit,
                 t_total=T):
    n = t_total * B
    x = np.asarray(x, np.float32).reshape(t_total, B, OBS)
    done = np.asarray(done, np.float32).reshape(t_total, B)
    action = np.asarray(action, np.int32).reshape(t_total, B)
    c = np.ascontiguousarray

    whd = np.concatenate([np.asarray(Wact), np.asarray(Wcrit)], axis=1)
    bhd = np.concatenate([np.asarray(bact), np.asarray(bcrit)])
    branch = {
        "a": dict(W1=W1a, b1=b1a, W2=W2a, b2=b2a, Wih=Wih_a, Whh=Whh_a,
                  bih=bih_a, bhh=bhh_a, Whd=whd, bhd=bhd, h0=h0a, c0=c0a),
        "c": dict(W1=W1c, b1=b1c, W2=W2c, b2=b2c, Wih=Wih_c, Whh=Whh_c,
                  bih=bih_c, bhh=bhh_c, Whd=whd, bhd=bhd, h0=h0c, c0=c0c),
    }
    in_maps = []
    for core in range(NCORES):
        br = branch["a" if core < 4 else "c"]
        e0 = 64 * (core % 4)
        xs = x[:, e0:e0 + BS].reshape(t_total * BS, OBS)
        ds = done[:, e0:e0 + BS].reshape(t_total * BS)
        As = action[:, e0:e0 + BS].reshape(t_total * BS)
        nch = (t_total * BS) // 128
        in_maps.append({
            "xT": c(xs.T.astype(np.float32)),
            "done": c(ds.astype(np.float32)),
            "actT": c(As.reshape(nch, 128).T.astype(np.int32)),
            "h0T": c(np.asarray(br["h0"], np.float32)[e0:e0 + BS].T),
            "c0T": c(np.asarray(br["c0"], np.float32)[e0:e0 + BS].T),
            "W1": c(np.asarray(br["W1"], np.float32)),
            "b1": c(np.asarray(br["b1"], np.float32)),
            "W2": c(np.asarray(br["W2"], np.float32)),
            "b2": c(np.asarray(br["b2"], np.float32)),
            "Wih": c(np.asarray(br["Wih"], np.float32)),
            "Whh": c(np.asarray(br["Whh"], np.float32)),
            "bih": c(np.asarray(br["bih"], np.float32)),
            "bhh": c(np.asarray(br["bhh"], np.float32)),
            "Whd": c(np.asarray(br["Whd"], np.float32)),
            "bhd": c(np.asarray(br["bhd"], np.float32)),
        })
    return in_maps


def assemble(results, t_total=T):
    n = t_total * B
    logp = np.zeros((t_total, B), np.float32)
    entropy = np.zeros((t_total, B), np.float32)
    value = np.zeros((t_total, B), np.float32)
    hTa = np.zeros((B, H), np.float32)
    cTa = np.zeros((B, H), np.float32)
    hTc = np.zeros((B, H), np.float32)
    cTc = np.zeros((B, H), np.float32)
    for core in range(NCORES):
        r = results[core]
        e0 = 64 * (core % 4)
        hT = r["hT_o"].reshape(H, BS).T   # [BS, H]
        cT = r["cT_o"].reshape(H, BS).T
        if core < 4:
            logp[:, e0:e0 + BS] = r["logp_o"].T.reshape(t_total, BS)
            entropy[:, e0:e0 + BS] = r["ent_o"].T.reshape(t_total, BS)
            hTa[e0:e0 + BS] = hT
            cTa[e0:e0 + BS] = cT
        else:
            value[:, e0:e0 + BS] = r["val_o"].T.reshape(t_total, BS)
            hTc[e0:e0 + BS] = hT
            cTc[e0:e0 + BS] = cT
    return (logp.reshape(n), entropy.reshape(n), value.reshape(n, 1),
            hTa, cTa, hTc, cTc)


def kernel(**inputs):
    nc = _get_nc()
    in_maps = make_in_maps(**inputs)
    res = run_bass_kernel_spmd(nc, in_maps, list(range(NCORES)))
    return assemble(res.results)
